# revision 6
# baseline (speedup 1.0000x reference)
"""Trainium2 Bass kernel for the distributed DCRNN (gnn_message_passing) problem.

Self-contained: host-side preprocessing (node sharding, degree-sorted
destination grids, gather index plumbing) + an SPMD Bass/Tile kernel that
runs on 8 NeuronCores via run_bass_kernel_spmd.
"""

from contextlib import ExitStack

import numpy as np

import concourse.bass as bass
import concourse.bacc as bacc
import concourse.mybir as mybir
import concourse.tile as tile
from concourse.masks import make_identity

P = 128
CH = 16
FILT = 64


# ---------------------------------------------------------------------------
# host-side preprocessing (index plumbing only; no reference arithmetic)
# ---------------------------------------------------------------------------

def chunk_plan(D, max_width=192):
    """Split tiles into gather chunks (contiguous tiles, bounded total width)
    and equal-width runs within each chunk.
    Returns list of chunks: (t_lo, t_hi, off_lo, off_hi, runs) with
    runs = [(t_lo, t_hi, D)]."""
    T = len(D)
    off = np.concatenate([[0], np.cumsum(D)]).astype(np.int64)
    chunks = []
    t = 0
    while t < T:
        t0 = t
        w = 0
        while t < T and (w + D[t] <= max_width or t == t0):
            w += D[t]
            t += 1
        runs = []
        r = t0
        while r < t:
            r0 = r
            while r < t and D[r] == D[r0]:
                r += 1
            runs.append((r0, r, int(D[r0])))
        chunks.append((t0, t, int(off[t0]), int(off[t]), runs))
    return chunks


def preprocess(x, edge_index, edge_weight, n_cores=8, sort_degrees=True):
    N = x.shape[0]
    E = edge_index.shape[1]
    NPC = N // n_cores
    T = (NPC + P - 1) // P
    NL = P * T
    row = np.ascontiguousarray(edge_index[0]).astype(np.int64)
    col = np.ascontiguousarray(edge_index[1]).astype(np.int64)
    w = np.ascontiguousarray(edge_weight).astype(np.float32)

    cnt_in = np.bincount(col, minlength=N)
    cnt_out = np.bincount(row, minlength=N)
    if sort_degrees:
        # deal nodes to cores by global in-degree rank (rank % n_cores): all
        # cores then share identical per-tile degree profiles, so the
        # max-over-cores tile-width padding of grid A vanishes
        g_order = np.argsort(cnt_in, kind="stable")
        cores = np.empty(N, dtype=np.int64)
        cores[g_order] = np.arange(N) % n_cores
    else:
        cores = np.arange(N) // NPC

    def make_perm(cnt):
        perm = np.full((n_cores, NL), -1, dtype=np.int64)
        pos = np.empty(N, dtype=np.int64)
        for k in range(n_cores):
            nodes = np.where(cores == k)[0]
            order = np.argsort(cnt[nodes], kind="stable") if sort_degrees \
                else np.arange(NPC)
            perm[k, :NPC] = nodes[order]
            pos[nodes[order]] = np.arange(NPC)
        return perm, pos

    permA, posA = make_perm(cnt_in)
    permB, posB = make_perm(cnt_out)

    def tile_widths(perm, cnt):
        D = np.zeros(T, dtype=np.int64)
        for k in range(n_cores):
            c = np.where(perm[k] >= 0, cnt[np.maximum(perm[k], 0)], 0)
            D = np.maximum(D, c.reshape(T, P).max(axis=1))
        return np.maximum(D, 1)

    DA = tile_widths(permA, cnt_in)
    DB = tile_widths(permB, cnt_out)
    offA = np.concatenate([[0], np.cumsum(DA)]).astype(np.int64)
    offB = np.concatenate([[0], np.cumsum(DB)]).astype(np.int64)
    WA, WB = int(offA[-1]), int(offB[-1])

    jA = posA
    tA, pA = jA // P, jA % P
    table_row = ((cores * P + pA) * T + tA).astype(np.int64)

    jpad = NPC
    if NPC < NL:
        pad_row = (np.arange(n_cores) * P + (jpad % P)) * T + (jpad // P)
    else:
        pad_row = np.zeros(n_cores, dtype=np.int64)

    def build_grid(dest, src, pos, off, W):
        idx = np.zeros((n_cores, P, W), dtype=np.int32)
        wg = np.zeros((n_cores, P, W), dtype=np.float32)
        for k in range(n_cores):
            idx[k, :, :] = pad_row[k]
        k_e = cores[dest]
        j_e = pos[dest]
        t_e, p_e = j_e // P, j_e % P
        order = np.argsort(dest, kind="stable")
        ds = dest[order]
        start = np.concatenate([[0], np.cumsum(np.bincount(ds, minlength=N))])[ds]
        s_e = np.empty(E, dtype=np.int64)
        s_e[order] = np.arange(E) - start
        wcol = off[t_e] + s_e
        idx[k_e, p_e, wcol] = table_row[src]
        wg[k_e, p_e, wcol] = w
        return idx, wg

    idxA, wgA = build_grid(col, row, posA, offA, WA)
    idxB, _wgB_sigma = build_grid(row, col, posB, offB, WB)
    # degree grid for deg_out in pi order directly (weights are host data, so
    # the looser pi-tile padding costs no gather descriptors)
    DBd = tile_widths(permA, cnt_out)
    offBd = np.concatenate([[0], np.cumsum(DBd)]).astype(np.int64)
    WBd = int(offBd[-1])
    _idxBd, wgBd = build_grid(row, col, posA, offBd, WBd)

    jB = posB
    sig_row = ((jB % P) * T + jB // P).astype(np.int64)
    perm_idx = np.zeros((n_cores, P, T), dtype=np.int32)
    scat_idx = np.zeros((n_cores, P, T), dtype=np.int32)
    pad_pi_row = (jpad % P) * T + jpad // P
    for k in range(n_cores):
        pk = permA[k]
        rows = np.where(pk >= 0, sig_row[np.maximum(pk, 0)], 0)
        perm_idx[k] = rows.reshape(T, P).T
        # sigma position (p', t') -> pi DRAM row (p_pi*T + t_pi) of its node
        pkB = permB[k]
        pi_row = np.full(NL, pad_pi_row, dtype=np.int64)
        validB = pkB >= 0
        jA_of = posA[np.maximum(pkB, 0)]
        pi_row[validB] = ((jA_of % P) * T + jA_of // P)[validB]
        scat_idx[k] = pi_row.reshape(T, P).T

    x_grid = np.zeros((n_cores, P, T, CH), dtype=np.float32)
    xT = np.zeros((n_cores, CH, NL), dtype=np.float32)
    for k in range(n_cores):
        pk = permA[k]
        valid = pk >= 0
        xg = np.zeros((NL, CH), dtype=np.float32)
        xg[valid] = x[pk[valid]]
        x_grid[k] = xg.reshape(T, P, CH).transpose(1, 0, 2)
        xT[k] = xg.T

    assert NPC < NL, "need at least one pad slot per core for zero gather rows"
    cfg = dict(
        N=N, E=E, NPC=NPC, T=T, NL=NL, WA=WA, WB=WB, WBd=WBd, n_cores=n_cores,
        chunksA=chunk_plan(DA), chunksB=chunk_plan(DB), chunksBd=chunk_plan(DBd, 10**9),
    )
    arrays = dict(
        idxA=idxA, wgA=wgA, idxB=idxB, wgB=wgBd, perm_idx=perm_idx,
        scat_idx=scat_idx, x_grid=x_grid, xT=xT, permA=permA,
    )
    return cfg, arrays


def make_in_maps(cfg, arrays, w_z, b_z, w_h, b_h, lin_w, lin_b):
    """A^T row layout: [x^T (0:16) | zeros (16:32) | TxO^T (32:48) | TxI^T (48:64)].
    Wcat rows match; rows 16:32 are zero (contraction-dim padding is free)."""
    n_cores = cfg["n_cores"]
    w_id0 = np.concatenate([w_z[0, 0, :CH], w_h[0, 0, :CH]], axis=1).astype(np.float32)
    w_id1 = np.concatenate([w_z[1, 0, :CH], w_h[1, 0, :CH]], axis=1).astype(np.float32)
    w_dif = np.concatenate(
        [np.concatenate([w_z[0, 1, :CH], w_h[0, 1, :CH]], axis=1),
         np.concatenate([w_z[1, 1, :CH], w_h[1, 1, :CH]], axis=1)],
        axis=0).astype(np.float32)
    bias = np.concatenate([b_z, b_h]).astype(np.float32).reshape(P, 1)
    in_maps = []
    for k in range(n_cores):
        in_maps.append({
            "x_grid": np.ascontiguousarray(arrays["x_grid"][k]),
            "xT": np.ascontiguousarray(arrays["xT"][k]),
            "idxA": np.ascontiguousarray(arrays["idxA"][k]),
            "idxB": np.ascontiguousarray(arrays["idxB"][k]),
            "wgA": np.ascontiguousarray(arrays["wgA"][k]),
            "wgB": np.ascontiguousarray(arrays["wgB"][k]),
            "perm_idx": np.ascontiguousarray(arrays["perm_idx"][k]),
            "w_id0": w_id0, "w_id1": w_id1, "w_dif": w_dif,
            "bias": bias,
            "lin_w": lin_w.astype(np.float32),
            "lin_b": lin_b.astype(np.float32).reshape(1, 1),
        })
    return in_maps


def postprocess(cfg, arrays, results):
    """results[k]['out'] is [1, NL]; scatter back to [N, 1] full output."""
    N, NL = cfg["N"], cfg["NL"]
    out = np.zeros((N, 1), dtype=np.float32)
    for k in range(cfg["n_cores"]):
        o = np.asarray(results[k]["out"]).reshape(NL)
        pk = arrays["permA"][k]
        valid = pk >= 0
        out[pk[valid], 0] = o[valid]
    return out


# ---------------------------------------------------------------------------
# device kernel
# ---------------------------------------------------------------------------

def build_kernel(cfg, debug=False):
    T, NL, WA, WB = cfg["T"], cfg["NL"], cfg["WA"], cfg["WB"]
    n_cores = cfg["n_cores"]
    NT = n_cores * NL
    f32 = mybir.dt.float32
    i32 = mybir.dt.int32

    nc = bacc.Bacc(num_swdge_queues=4)
    dbg = {}
    if debug:
        for name, shape in (("d_degI", [P, T]), ("d_degO", [P, T]),
                            ("d_xs", [P, T, 2 * CH]), ("d_table", [n_cores * NL, 2 * CH]),
                            ("d_TxC", [P, T, 2 * CH]), ("d_AT", [FILT, NL]),
                            ("d_Wcat", [FILT, P])):
            dbg[name] = nc.declare_dram_parameter(name, shape, f32, isOutput=True)

    x_grid_p = nc.declare_dram_parameter("x_grid", [P, T, CH], f32, isOutput=False)
    xT_p = nc.declare_dram_parameter("xT", [CH, NL], f32, isOutput=False)
    idxA_p = nc.declare_dram_parameter("idxA", [P, WA], i32, isOutput=False)
    idxB_p = nc.declare_dram_parameter("idxB", [P, WB], i32, isOutput=False)
    wgA_p = nc.declare_dram_parameter("wgA", [P, WA], f32, isOutput=False)
    wgB_p = nc.declare_dram_parameter("wgB", [P, cfg["WBd"]], f32, isOutput=False)
    perm_p = nc.declare_dram_parameter("perm_idx", [P, T], i32, isOutput=False)
    w_id0_p = nc.declare_dram_parameter("w_id0", [CH, P], f32, isOutput=False)
    w_id1_p = nc.declare_dram_parameter("w_id1", [CH, P], f32, isOutput=False)
    w_dif_p = nc.declare_dram_parameter("w_dif", [2 * CH, P], f32, isOutput=False)
    bias_p = nc.declare_dram_parameter("bias", [P, 1], f32, isOutput=False)
    lin_w_p = nc.declare_dram_parameter("lin_w", [FILT, 1], f32, isOutput=False)
    lin_b_p = nc.declare_dram_parameter("lin_b", [1, 1], f32, isOutput=False)
    out_p = nc.declare_dram_parameter("out", [1, NL], f32, isOutput=True)

    bounceO = nc.dram_tensor("bounceO", [NL, CH], f32)
    bounceI = nc.dram_tensor("bounceI", [NL, CH], f32)
    tableO = nc.dram_tensor("tableO", [NT, CH], f32, addr_space="Shared")
    tableI = nc.dram_tensor("tableI", [NT, CH], f32, addr_space="Shared")
    txis_d = nc.dram_tensor("txis", [NL, CH], f32)

    replica_groups = [list(range(n_cores))]

    with ExitStack() as ctx:
        tc = ctx.enter_context(tile.TileContext(nc))
        persist = ctx.enter_context(tc.tile_pool(name="persist", bufs=1))
        work = ctx.enter_context(tc.tile_pool(name="work", bufs=2))
        gpool = ctx.enter_context(tc.tile_pool(name="gpool", bufs=3))
        psum = ctx.enter_context(tc.tile_pool(name="psum", bufs=2, space="PSUM"))
        psum_pre = ctx.enter_context(tc.tile_pool(name="psum_pre", bufs=2, space="PSUM"))

        # ---- persistent tiles & input DMAs ----
        xg = persist.tile([P, T, CH], f32)
        AT = persist.tile([FILT, NL], f32)
        idxA_t = persist.tile([P, WA], i32)
        idxB_t = persist.tile([P, WB], i32)
        wgA_t = persist.tile([P, WA], f32)
        wgB_t = persist.tile([P, cfg["WBd"]], f32)
        perm_t = persist.tile([P, T], i32)
        Wcat = persist.tile([FILT, P], f32)
        w_id0_t = persist.tile([CH, P], f32)
        w_id1_t = persist.tile([CH, P], f32)
        bias_t = persist.tile([P, 1], f32)
        bias_h = persist.tile([P, 1], f32)
        lin_w_t = persist.tile([FILT, 1], f32)
        lin_b_t = persist.tile([1, 1], f32)
        ident = persist.tile([P, P], f32)

        nc.vector.memset(AT[0:2 * CH, :], 0.0)
        nc.vector.memset(Wcat[0:2 * CH, :], 0.0)
        nc.sync.dma_start(out=wgA_t[:], in_=wgA_p[:])
        nc.sync.dma_start(out=wgB_t[:], in_=wgB_p[:])
        nc.sync.dma_start(out=xg[:], in_=x_grid_p[:])
        nc.sync.dma_start(out=AT[0:CH, :], in_=xT_p[:])
        nc.sync.dma_start(out=idxA_t[:], in_=idxA_p[:])
        nc.sync.dma_start(out=idxB_t[:], in_=idxB_p[:])
        nc.sync.dma_start(out=perm_t[:], in_=perm_p[:])
        nc.sync.dma_start(out=w_id0_t[:], in_=w_id0_p[:])
        nc.sync.dma_start(out=w_id1_t[:], in_=w_id1_p[:])
        nc.sync.dma_start(out=Wcat[2 * CH:4 * CH, :], in_=w_dif_p[:])
        nc.sync.dma_start(out=bias_t[:], in_=bias_p[:])
        nc.sync.dma_start(out=lin_w_t[:], in_=lin_w_p[:])
        nc.sync.dma_start(out=lin_b_t[:], in_=lin_b_p[:])
        make_identity(nc, ident[:])

        nc.vector.tensor_add(out=Wcat[0:CH, :], in0=w_id0_t[:], in1=w_id1_t[:])
        # bias halves: Z-part scaled by 0.5 for the tanh-based sigmoid
        nc.vector.tensor_scalar_mul(out=bias_h[0:FILT, :], in0=bias_t[0:FILT, :],
                                    scalar1=0.5)
        nc.vector.tensor_copy(out=bias_h[FILT:P, :], in_=bias_t[FILT:P, :])

        # ---- phase 1: degrees (both in pi order; no permute needed) ----
        degI = persist.tile([P, T], f32)
        degO = persist.tile([P, T, 1], f32)
        for wg_t, deg, chunks in ((wgA_t, degI[:, :], cfg["chunksA"]),
                                  (wgB_t, degO[:, :, 0], cfg["chunksBd"])):
            for (t0, t1, o0, o1, runs) in chunks:
                ro = o0
                for (r0, r1, D) in runs:
                    nt = r1 - r0
                    nc.vector.tensor_reduce(
                        out=deg[:, r0:r1],
                        in_=wg_t[:, ro:ro + nt * D].rearrange(
                            "p (t d) -> p t d", t=nt),
                        axis=mybir.AxisListType.X, op=mybir.AluOpType.add)
                    ro += nt * D

        rin = persist.tile([P, T], f32)
        rout = persist.tile([P, T], f32)
        nc.vector.tensor_scalar_max(out=rin[:], in0=degI[:], scalar1=1e-30)
        nc.vector.reciprocal(out=rin[:], in_=rin[:])
        nc.vector.tensor_scalar_max(out=rout[:], in0=degO[:, :, 0], scalar1=1e-30)
        nc.vector.reciprocal(out=rout[:], in_=rout[:])

        # ---- phase 2: xs tables + two AllGathers ----
        # xs_i first: the B-direction storm runs first and only needs tableI;
        # the tableO AllGather then hides under the running B storm.
        xsI = persist.tile([P, T, CH], f32)
        xsO = persist.tile([P, T, CH], f32)
        nc.vector.tensor_tensor(out=xsI[:], in0=xg[:],
                                in1=rin[:].to_broadcast([P, T, CH]),
                                op=mybir.AluOpType.mult)
        nc.sync.dma_start(out=bounceI[:], in_=xsI[:])
        nc.gpsimd.collective_compute(
            "AllGather", mybir.AluOpType.bypass,
            replica_groups=replica_groups,
            ins=[bounceI[:]], outs=[tableI[:]])
        nc.vector.tensor_tensor(out=xsO[:], in0=xg[:],
                                in1=rout[:].to_broadcast([P, T, CH]),
                                op=mybir.AluOpType.mult)
        nc.sync.dma_start(out=bounceO[:], in_=xsO[:])
        nc.gpsimd.collective_compute(
            "AllGather", mybir.AluOpType.bypass,
            replica_groups=replica_groups,
            ins=[bounceO[:]], outs=[tableO[:]])

        # ---- phase 3: gathers + segmented reduces ----
        # HW indirect DMA semantics: one descriptor per partition, reading
        # out.free_size contiguous elements from idx[p, 0]*coef. So each
        # instruction gathers one grid column: G[:, w, :] = table[idxX[:, w]].
        # Q7 descriptor generation runs at ~8.6ns/desc *per SWDGE queue*;
        # striping columns round-robin across the 4 queues generates (and
        # drains) ~4 columns concurrently.
        # TxC channels 0:16 = TxO (pi order), 16:32 = TxI (permuted in below)
        TxC = persist.tile([P, T, 2 * CH], f32)
        TxIs = persist.tile([P, T, CH], f32)
        def q_indirect(out, in_, in_offset, w):
            inst = nc.gpsimd.indirect_dma_start(
                out=out, out_offset=None, in_=in_, in_offset=in_offset)
            inst.ins.queue = f"qPoolDynamic{w % 4 or ''}"
            return inst
        def storm(idx_t, Tx, coff, chunks, table):
            for (t0, t1, o0, o1, runs) in chunks:
                wchunk = o1 - o0
                G = gpool.tile([P, 192, CH], f32, tag="gbuf")
                for w in range(wchunk):
                    q_indirect(G[:, w, :], table[:],
                               bass.IndirectOffsetOnAxis(
                                   ap=idx_t[:, o0 + w:o0 + w + 1], axis=0), w)
                ro = 0
                for (r0, r1, D) in runs:
                    nt = r1 - r0
                    out_ap = (Tx[:, r0:r1, 0:CH] if coff is not None
                              else Tx[:, r0:r1, :])
                    nc.vector.tensor_reduce(
                        out=out_ap,
                        in_=G[:, ro:ro + nt * D, :].rearrange(
                            "p (t d) c -> p t c d", t=nt),
                        axis=mybir.AxisListType.X, op=mybir.AluOpType.add)
                    ro += nt * D
        storm(idxB_t, TxIs, None, cfg["chunksB"], tableI)
        storm(idxA_t, TxC, 0, cfg["chunksA"], tableO)
        # permute TxI sigma->pi through DRAM, landing in TxC channels 16:32
        nc.sync.dma_start(out=txis_d[:], in_=TxIs[:])
        for t in range(T):
            q_indirect(TxC[:, t, CH:2 * CH], txis_d[:],
                       bass.IndirectOffsetOnAxis(
                           ap=perm_t[:, t:t + 1], axis=0), t)

        # ---- phase 4: transposes into AT rows 32:64 ----
        # 4 tiles per transpose: out rows 32*i:32*i+32 = tile (g0+i) [TxO|TxI]
        for g0 in range(0, T, 4):
            nt = min(4, T - g0)
            ps = psum.tile([P, P], f32, tag="tps")
            nc.tensor.transpose(
                out=ps[0:nt * 2 * CH, :],
                in_=TxC[:, g0:g0 + nt, :].rearrange("p t c -> p (t c)"),
                identity=ident[:])
            for i in range(nt):
                nc.scalar.copy(
                    out=AT[2 * CH:4 * CH, (g0 + i) * P:(g0 + i + 1) * P],
                    in_=ps[i * 2 * CH:(i + 1) * 2 * CH, :])

        # ---- phase 5: epilogue ----
        out_sb = persist.tile([1, NL], f32)
        CW = 512
        nchunks = (NL + CW - 1) // CW
        for c in range(nchunks):
            lo = c * CW
            w = min(CW, NL - lo)
            pre = psum_pre.tile([P, CW], f32, tag="pre")
            nc.tensor.matmul(out=pre[:, 0:w], lhsT=Wcat[:], rhs=AT[:, lo:lo + w],
                             start=True, stop=True)
            z = work.tile([FILT, CW], f32, tag="z")
            ht = work.tile([FILT, CW], f32, tag="ht")
            # sigmoid(x) = 0.5*tanh(0.5*x) + 0.5  (single ACT table)
            nc.scalar.activation(out=z[:, 0:w], in_=pre[0:FILT, 0:w],
                                 func=mybir.ActivationFunctionType.Tanh,
                                 bias=bias_h[0:FILT, :], scale=0.5)
            nc.scalar.activation(out=ht[:, 0:w], in_=pre[FILT:P, 0:w],
                                 func=mybir.ActivationFunctionType.Tanh,
                                 bias=bias_h[FILT:P, :], scale=1.0)
            nc.vector.tensor_scalar(out=z[:, 0:w], in0=z[:, 0:w],
                                    scalar1=0.5, scalar2=0.5,
                                    op0=mybir.AluOpType.mult,
                                    op1=mybir.AluOpType.add)
            h = work.tile([FILT, CW], f32, tag="h")
            nc.vector.tensor_mul(out=h[:, 0:w], in0=z[:, 0:w], in1=ht[:, 0:w])
            nc.vector.tensor_tensor(out=h[:, 0:w], in0=ht[:, 0:w], in1=h[:, 0:w],
                                    op=mybir.AluOpType.subtract)
            nc.vector.tensor_scalar_max(out=h[:, 0:w], in0=h[:, 0:w], scalar1=0.0)
            ps2 = psum.tile([1, CW], f32, tag="ps2")
            nc.tensor.matmul(out=ps2[:, 0:w], lhsT=lin_w_t[:], rhs=h[:, 0:w],
                             start=True, stop=True)
            nc.vector.tensor_scalar_add(out=out_sb[:, lo:lo + w], in0=ps2[:, 0:w],
                                        scalar1=lin_b_t[0:1, :])
        nc.sync.dma_start(out=out_p[:], in_=out_sb[:])

        if debug:
            nc.sync.dma_start(out=dbg["d_degI"][:], in_=degI[:])
            nc.sync.dma_start(out=dbg["d_degO"][:], in_=degO[:, :, 0])
            nc.sync.dma_start(out=dbg["d_TxC"][:], in_=TxC[:])
            nc.sync.dma_start(out=dbg["d_AT"][:], in_=AT[:])
            nc.sync.dma_start(out=dbg["d_Wcat"][:], in_=Wcat[:])

    nc.compile()
    return nc


# ---------------------------------------------------------------------------
# harness entry point
# ---------------------------------------------------------------------------

_CACHE = {}


def kernel(x, edge_index, edge_weight, w_z, b_z, w_r, b_r, w_h, b_h, lin_w, lin_b):
    """Distributed DCRNN forward on 8 TRN2 NeuronCores.

    Takes full unsharded inputs, returns the full [N, 1] float32 output.
    (w_r/b_r are dead inputs: H0 = 0 makes the reset gate a no-op.)
    """
    from concourse.bass_utils import run_bass_kernel_spmd

    x = np.ascontiguousarray(np.asarray(x, dtype=np.float32))
    cfg, arrays = preprocess(x, np.asarray(edge_index), np.asarray(edge_weight),
                             n_cores=8)
    in_maps = make_in_maps(cfg, arrays, np.asarray(w_z, np.float32),
                           np.asarray(b_z, np.float32),
                           np.asarray(w_h, np.float32),
                           np.asarray(b_h, np.float32),
                           np.asarray(lin_w, np.float32),
                           np.asarray(lin_b, np.float32))
    key = (cfg["N"], cfg["E"], cfg["WA"], cfg["WB"], cfg["WBd"],
           tuple(tuple(c[:4]) for c in cfg["chunksA"]),
           tuple(tuple(c[:4]) for c in cfg["chunksB"]))
    nc = _CACHE.get(key)
    if nc is None:
        nc = build_kernel(cfg)
        _CACHE[key] = nc
    res = run_bass_kernel_spmd(nc, in_maps, core_ids=list(range(8)))
    return postprocess(cfg, arrays, res.results)



# revision 12
# speedup vs baseline: 2.0023x; 2.0023x over previous
"""Trainium2 Bass kernel for the distributed DCRNN (gnn_message_passing) problem.

Self-contained: host-side preprocessing (node sharding, 2D degree-sorted
destination grids, int16 gather index plumbing) + an SPMD Bass/Tile kernel
running on 8 NeuronCores via run_bass_kernel_spmd.

Gather architecture: the per-edge source gathers use InstDMAGatherAnt
(custom SWDGE ucode) striped across 4 SWDGE queues. Descriptor generation
runs at ~8.6ns/desc per queue, so 4 queues gather ~4 columns concurrently.
GatherAnt constraints honored here:
  - indices are int16 (< 32768), so the 50176-row xs table is addressed as
    two base-offset views split at the core-4 boundary (row 25088); each
    destination grid is built as two slot regions (low/high source half)
    with per-tile widths kept tight by a (total, d_low) 2D node sort.
  - the table row stride must be a multiple of 256B, so the AllGather'd
    [NT, 16] f32 tables are restrided on device into [NT, 64] f32 buffers
    (only [:, 0:16] meaningful) via an SBUF bounce.
  - num_idxs <= 1024 per instruction (SWDGE ring capacity), i.e. 8 grid
    columns per gather instruction.
"""

from contextlib import ExitStack

import numpy as np

import concourse.bass as bass
import concourse.bacc as bacc
import concourse.mybir as mybir
import concourse.tile as tile
from concourse.masks import make_identity

P = 128
CH = 16
FILT = 64
CUT_CORES = 4          # table rows of cores 0..3 form the "low" half
WMAX = 192             # gather-chunk width (columns) held in SBUF at once
COLS_PER_INST = 8      # 8 cols * 128 = 1024 idxs = SWDGE ring capacity


# ---------------------------------------------------------------------------
# host-side preprocessing (index plumbing only; no reference arithmetic)
# ---------------------------------------------------------------------------

def chunk_plan(D, max_width=WMAX):
    """Split tiles into gather chunks (contiguous tiles, bounded total width)
    and equal-width runs within each chunk."""
    T = len(D)
    off = np.concatenate([[0], np.cumsum(D)]).astype(np.int64)
    chunks = []
    t = 0
    while t < T:
        t0 = t
        w = 0
        while t < T and (w + D[t] <= max_width or t == t0):
            w += D[t]
            t += 1
        runs = []
        r = t0
        while r < t:
            r0 = r
            while r < t and D[r] == D[r0]:
                r += 1
            runs.append((r0, r, int(D[r0])))
        chunks.append((t0, t, int(off[t0]), int(off[t]), runs))
    return chunks


def wrap_idx(unw):
    """Wrap a flat int index list into GatherAnt's [128, n/16] int16 layout:
    index i lives at partition i%16 (replicated mod 16), column i//16."""
    n = len(unw)
    assert n % 16 == 0
    S = n // 16
    cols = np.asarray(unw, dtype=np.int64).reshape(S, 16)
    tilep = np.tile(cols.T, (8, 1))          # [128, S]
    assert tilep.max() < 32768 and tilep.min() >= 0
    return np.ascontiguousarray(tilep.astype(np.int16))


def build_region_grids(dst, src, deal, order_pos, table_row, cut_row,
                       pad_low, pad_high, n_cores, T, NPC, NL):
    """Build the two slot-region grids for one direction.

    dst/src: per-edge endpoint node ids (dst sharded by `deal`, grid position
    from `order_pos`). Returns (D0, D1, unw0, unw1) where unw* are per-core
    flat gather lists (slot i = column i//128, partition i%128) holding
    rebased table rows of the sources, and D* are shared per-tile widths.
    """
    N = len(deal)
    E = len(dst)
    src_row = table_row[src]
    low = src_row < cut_row

    d_low = np.bincount(dst[low], minlength=N)
    d_high = np.bincount(dst[~low], minlength=N)

    def widths(dcnt):
        D = np.zeros(T, dtype=np.int64)
        for k in range(n_cores):
            nodes = np.where(deal == k)[0]
            c = np.zeros(NL, dtype=np.int64)
            c[order_pos[nodes]] = dcnt[nodes]
            D = np.maximum(D, c.reshape(T, P).max(axis=1))
        return np.maximum(D, 1)

    D0 = widths(d_low)
    D1 = widths(d_high)
    off0 = np.concatenate([[0], np.cumsum(D0)]).astype(np.int64)
    off1 = np.concatenate([[0], np.cumsum(D1)]).astype(np.int64)
    W0, W1 = int(off0[-1]), int(off1[-1])

    unw0 = np.full((n_cores, W0 * P), pad_low, dtype=np.int64)
    unw1 = np.full((n_cores, W1 * P), pad_high - cut_row, dtype=np.int64)

    # slot position within each (dst, region): cumulative count per dst+region
    k_e = deal[dst]
    j_e = order_pos[dst]
    t_e, p_e = j_e // P, j_e % P
    # rank edges within (dst, regionflag)
    key = dst * 2 + (~low).astype(np.int64)
    order = np.argsort(key, kind="stable")
    ks = key[order]
    start = np.concatenate([[0], np.cumsum(np.bincount(ks, minlength=2 * N))])[ks]
    s_e = np.empty(E, dtype=np.int64)
    s_e[order] = np.arange(E) - start

    # region 0 slots
    m0 = low
    w0 = off0[t_e[m0]] + s_e[m0]
    unw0[k_e[m0], w0 * P + p_e[m0]] = src_row[m0]
    # region 1 slots (rebased)
    m1 = ~low
    w1 = off1[t_e[m1]] + s_e[m1]
    unw1[k_e[m1], w1 * P + p_e[m1]] = src_row[m1] - cut_row
    return D0, D1, unw0, unw1


def preprocess(x, edge_index, edge_weight, n_cores=8):
    N = x.shape[0]
    E = edge_index.shape[1]
    NPC = N // n_cores
    T = (NPC + P - 1) // P
    NL = P * T
    NT = n_cores * NL
    cut_row = CUT_CORES * NL
    row = np.ascontiguousarray(edge_index[0]).astype(np.int64)
    col = np.ascontiguousarray(edge_index[1]).astype(np.int64)
    w = np.ascontiguousarray(edge_weight).astype(np.float32)

    cnt_in = np.bincount(col, minlength=N)
    cnt_out = np.bincount(row, minlength=N)

    # --- node -> core deal (shared by A and B so the sigma->pi permute is
    # core-local). Two passes: refine the deal by (d_in_low, total_in) 2D
    # rank so all cores share tile profiles, then recompute d_low exactly.
    g_order = np.argsort(cnt_in, kind="stable")
    dealA = np.empty(N, dtype=np.int64)
    dealA[g_order] = np.arange(N) % n_cores
    low_src = dealA[row] < CUT_CORES
    d_low0 = np.bincount(col[low_src], minlength=N)
    order1 = np.lexsort((d_low0, cnt_in))
    dealA[order1] = np.arange(N) % n_cores

    # final pi order: within-core by (total_in, d_in_low) rank w.r.t. the
    # FINAL deal's cut (core < CUT_CORES, independent of within-core order)
    low_src = dealA[row] < CUT_CORES
    d_low = np.bincount(col[low_src], minlength=N)
    order2 = np.lexsort((d_low, cnt_in))
    posA = np.empty(N, dtype=np.int64)
    permA = np.full((n_cores, NL), -1, dtype=np.int64)
    for k in range(n_cores):
        nodes = order2[dealA[order2] == k]
        permA[k, :NPC] = nodes
        posA[nodes] = np.arange(NPC)
    table_row = (dealA * P + posA % P) * T + posA // P

    # --- B (sigma) order: same deal, within-core by (total_out, d_out_low)
    d_out_low = np.bincount(row[dealA[col] < CUT_CORES], minlength=N)
    orderB = np.lexsort((d_out_low, cnt_out))
    posB = np.empty(N, dtype=np.int64)
    permB = np.full((n_cores, NL), -1, dtype=np.int64)
    for k in range(n_cores):
        nodes = orderB[dealA[orderB] == k]
        permB[k, :NPC] = nodes
        posB[nodes] = np.arange(NPC)

    # pad rows (zero xs): position NPC of core 0 (low) / last core (high)
    assert NPC < NL, "need a pad slot per core"
    pad_low = (0 * P + NPC % P) * T + NPC // P
    pad_high = ((n_cores - 1) * P + NPC % P) * T + NPC // P
    assert pad_low < cut_row <= pad_high

    # --- gather grids (regions by source table-row half) ---
    D0A, D1A, unwA0, unwA1 = build_region_grids(
        col, row, dealA, posA, table_row, cut_row, pad_low, pad_high,
        n_cores, T, NPC, NL)
    D0B, D1B, unwB0, unwB1 = build_region_grids(
        row, col, dealA, posB, table_row, cut_row, pad_low, pad_high,
        n_cores, T, NPC, NL)

    # --- permute gather (TxI sigma -> pi): slot (p, t) of pi grid reads
    # sigma-layout DRAM row of that node: (posB%P)*T + posB//P
    sig_row = (posB % P) * T + posB // P
    unwP = np.zeros((n_cores, T * P), dtype=np.int64)
    pad_sig = (NPC % P) * T + NPC // P
    for k in range(n_cores):
        pk = permA[k]
        r = np.full(NL, pad_sig, dtype=np.int64)
        valid = pk >= 0
        r[valid] = sig_row[pk[valid]]
        # slot i = t*128 + p ; node at pi pos (p, t) has j = t*P? no: j = posA
        # pi grid (p, t): j = posA = t? j%P=p, j//P=t -> i = t*128+p = j//P*128+j%P
        j = np.arange(NL)
        i = (j // P) * P + (j % P)
        out = np.empty(NL, dtype=np.int64)
        out[i] = r[j]
        unwP[k] = out

    # --- degree weight grids (host data; in pi order for both directions) ---
    def tile_widths_for(perm, cnt):
        D = np.zeros(T, dtype=np.int64)
        for k in range(n_cores):
            c = np.where(perm[k] >= 0, cnt[np.maximum(perm[k], 0)], 0)
            D = np.maximum(D, c.reshape(T, P).max(axis=1))
        return np.maximum(D, 1)

    DAd = tile_widths_for(permA, cnt_in)
    DBd = tile_widths_for(permA, cnt_out)
    offAd = np.concatenate([[0], np.cumsum(DAd)]).astype(np.int64)
    offBd = np.concatenate([[0], np.cumsum(DBd)]).astype(np.int64)
    WAd, WBd = int(offAd[-1]), int(offBd[-1])

    def build_weight_grid(dest, off, W):
        wg = np.zeros((n_cores, P, W), dtype=np.float32)
        k_e = dealA[dest]
        j_e = posA[dest]
        t_e, p_e = j_e // P, j_e % P
        order = np.argsort(dest, kind="stable")
        ds = dest[order]
        start = np.concatenate([[0], np.cumsum(np.bincount(ds, minlength=N))])[ds]
        s_e = np.empty(E, dtype=np.int64)
        s_e[order] = np.arange(E) - start
        wcol = off[t_e] + s_e
        wg[k_e, p_e, wcol] = w
        return wg

    wgA = build_weight_grid(col, offAd, WAd)
    wgB = build_weight_grid(row, offBd, WBd)

    # --- x in pi order ---
    x_grid = np.zeros((n_cores, P, T, CH), dtype=np.float32)
    xT = np.zeros((n_cores, CH, NL), dtype=np.float32)
    for k in range(n_cores):
        pk = permA[k]
        valid = pk >= 0
        xg = np.zeros((NL, CH), dtype=np.float32)
        xg[valid] = x[pk[valid]]
        x_grid[k] = xg.reshape(T, P, CH).transpose(1, 0, 2)
        xT[k] = xg.T

    cfg = dict(
        N=N, E=E, NPC=NPC, T=T, NL=NL, NT=NT, n_cores=n_cores,
        cut_row=cut_row,
        W0A=int(D0A.sum()), W1A=int(D1A.sum()),
        W0B=int(D0B.sum()), W1B=int(D1B.sum()),
        WAd=WAd, WBd=WBd,
        chunksA0=chunk_plan(D0A), chunksA1=chunk_plan(D1A),
        chunksB0=chunk_plan(D0B), chunksB1=chunk_plan(D1B),
        chunksAd=chunk_plan(DAd, 10 ** 9), chunksBd=chunk_plan(DBd, 10 ** 9),
    )
    arrays = dict(
        idxA0=np.stack([wrap_idx(unwA0[k]) for k in range(n_cores)]),
        idxA1=np.stack([wrap_idx(unwA1[k]) for k in range(n_cores)]),
        idxB0=np.stack([wrap_idx(unwB0[k]) for k in range(n_cores)]),
        idxB1=np.stack([wrap_idx(unwB1[k]) for k in range(n_cores)]),
        idxP=np.stack([wrap_idx(unwP[k]) for k in range(n_cores)]),
        wgA=wgA, wgB=wgB, x_grid=x_grid, xT=xT, permA=permA,
    )
    return cfg, arrays


def make_in_maps(cfg, arrays, w_z, b_z, w_h, b_h, lin_w, lin_b):
    """AT row layout: [x^T (0:16) | zeros (16:32) | TxO^T (32:48) | TxI^T
    (48:64)]. Wcat rows match; rows 16:32 are zero."""
    n_cores = cfg["n_cores"]
    w_id0 = np.concatenate([w_z[0, 0, :CH], w_h[0, 0, :CH]], axis=1).astype(np.float32)
    w_id1 = np.concatenate([w_z[1, 0, :CH], w_h[1, 0, :CH]], axis=1).astype(np.float32)
    w_dif = np.concatenate(
        [np.concatenate([w_z[0, 1, :CH], w_h[0, 1, :CH]], axis=1),
         np.concatenate([w_z[1, 1, :CH], w_h[1, 1, :CH]], axis=1)],
        axis=0).astype(np.float32)
    bias = np.concatenate([b_z, b_h]).astype(np.float32).reshape(P, 1)
    in_maps = []
    for k in range(n_cores):
        in_maps.append({
            "x_grid": np.ascontiguousarray(arrays["x_grid"][k]),
            "xT": np.ascontiguousarray(arrays["xT"][k]),
            "idxA0": np.ascontiguousarray(arrays["idxA0"][k]),
            "idxA1": np.ascontiguousarray(arrays["idxA1"][k]),
            "idxB0": np.ascontiguousarray(arrays["idxB0"][k]),
            "idxB1": np.ascontiguousarray(arrays["idxB1"][k]),
            "idxP": np.ascontiguousarray(arrays["idxP"][k]),
            "wgA": np.ascontiguousarray(arrays["wgA"][k]),
            "wgB": np.ascontiguousarray(arrays["wgB"][k]),
            "w_id0": w_id0, "w_id1": w_id1, "w_dif": w_dif,
            "bias": bias,
            "lin_w": lin_w.astype(np.float32),
            "lin_b": lin_b.astype(np.float32).reshape(1, 1),
        })
    return in_maps


def postprocess(cfg, arrays, results):
    N, NL = cfg["N"], cfg["NL"]
    out = np.zeros((N, 1), dtype=np.float32)
    for k in range(cfg["n_cores"]):
        o = np.asarray(results[k]["out"]).reshape(NL)
        pk = arrays["permA"][k]
        valid = pk >= 0
        out[pk[valid], 0] = o[valid]
    return out


# ---------------------------------------------------------------------------
# device kernel
# ---------------------------------------------------------------------------

def build_kernel(cfg):
    T, NL, NT = cfg["T"], cfg["NL"], cfg["NT"]
    cut_row = cfg["cut_row"]
    n_cores = cfg["n_cores"]
    f32 = mybir.dt.float32
    i16 = mybir.dt.int16

    nc = bacc.Bacc(num_swdge_queues=4)

    x_grid_p = nc.declare_dram_parameter("x_grid", [P, T, CH], f32, isOutput=False)
    xT_p = nc.declare_dram_parameter("xT", [CH, NL], f32, isOutput=False)
    SA0, SA1 = cfg["W0A"] * 8, cfg["W1A"] * 8
    SB0, SB1 = cfg["W0B"] * 8, cfg["W1B"] * 8
    idxA0_p = nc.declare_dram_parameter("idxA0", [P, SA0], i16, isOutput=False)
    idxA1_p = nc.declare_dram_parameter("idxA1", [P, SA1], i16, isOutput=False)
    idxB0_p = nc.declare_dram_parameter("idxB0", [P, SB0], i16, isOutput=False)
    idxB1_p = nc.declare_dram_parameter("idxB1", [P, SB1], i16, isOutput=False)
    idxP_p = nc.declare_dram_parameter("idxP", [P, T * 8], i16, isOutput=False)
    wgA_p = nc.declare_dram_parameter("wgA", [P, cfg["WAd"]], f32, isOutput=False)
    wgB_p = nc.declare_dram_parameter("wgB", [P, cfg["WBd"]], f32, isOutput=False)
    w_id0_p = nc.declare_dram_parameter("w_id0", [CH, P], f32, isOutput=False)
    w_id1_p = nc.declare_dram_parameter("w_id1", [CH, P], f32, isOutput=False)
    w_dif_p = nc.declare_dram_parameter("w_dif", [2 * CH, P], f32, isOutput=False)
    bias_p = nc.declare_dram_parameter("bias", [P, 1], f32, isOutput=False)
    lin_w_p = nc.declare_dram_parameter("lin_w", [FILT, 1], f32, isOutput=False)
    lin_b_p = nc.declare_dram_parameter("lin_b", [1, 1], f32, isOutput=False)
    out_p = nc.declare_dram_parameter("out", [1, NL], f32, isOutput=True)

    bounceO = nc.dram_tensor("bounceO", [NL, CH], f32)
    bounceI = nc.dram_tensor("bounceI", [NL, CH], f32)
    tableO = nc.dram_tensor("tableO", [NT, CH], f32, addr_space="Shared")
    tableI = nc.dram_tensor("tableI", [NT, CH], f32, addr_space="Shared")
    tableO64 = nc.dram_tensor("tableO64", [NT, 64], f32)
    tableI64 = nc.dram_tensor("tableI64", [NT, 64], f32)
    txis_d = nc.dram_tensor("txis", [NL, 64], f32)

    replica_groups = [list(range(n_cores))]

    def dma_gather_q(out_ap, in_ap, idxs_ap, num_idxs, queue_num):
        g = nc.gpsimd
        _in_ap = g.lower_ap_dma(in_ap, for_custom_bir_dma=True)
        _idxs_ap = g.lower_ap(idxs_ap)
        _out_ap = g.lower_ap(out_ap)
        return g.add_instruction(mybir.InstDMAGatherAnt(
            name=g.bass.get_next_instruction_name(),
            ins=[*_in_ap, _idxs_ap, g.lower_val_access(g.to_reg(num_idxs))],
            outs=[_out_ap],
            transpose=False, num_idxs=num_idxs, elem_size=CH,
            stride_bytes_256=1, gen_mode=0, single_packet=True,
            queue_num=queue_num, sbuf_tokens_per_rank=0,
            sbuf_free_dim_per_rank=0, sbuf_free_dim_pad_per_rank=0,
            sbuf_byte_offset=0))

    with ExitStack() as ctx:
        tc = ctx.enter_context(tile.TileContext(nc))
        persist = ctx.enter_context(tc.tile_pool(name="persist", bufs=1))
        work = ctx.enter_context(tc.tile_pool(name="work", bufs=2))
        gpool = ctx.enter_context(tc.tile_pool(name="gpool", bufs=3))
        rpool = ctx.enter_context(tc.tile_pool(name="rpool", bufs=1))
        psum = ctx.enter_context(tc.tile_pool(name="psum", bufs=2, space="PSUM"))
        psum_pre = ctx.enter_context(tc.tile_pool(name="psum_pre", bufs=2, space="PSUM"))

        # ---- persistent tiles & input DMAs ----
        xg = persist.tile([P, T, CH], f32)
        AT = persist.tile([FILT, NL], f32)
        idxA0_t = persist.tile([P, SA0], i16)
        idxA1_t = persist.tile([P, SA1], i16)
        idxB0_t = persist.tile([P, SB0], i16)
        idxB1_t = persist.tile([P, SB1], i16)
        idxP_t = persist.tile([P, T * 8], i16)
        wgA_t = persist.tile([P, cfg["WAd"]], f32)
        wgB_t = persist.tile([P, cfg["WBd"]], f32)
        Wcat = persist.tile([FILT, P], f32)
        w_id0_t = persist.tile([CH, P], f32)
        w_id1_t = persist.tile([CH, P], f32)
        bias_t = persist.tile([P, 1], f32)
        bias_h = persist.tile([P, 1], f32)
        lin_w_t = persist.tile([FILT, 1], f32)
        lin_b_t = persist.tile([1, 1], f32)
        ident = persist.tile([P, P], f32)

        nc.vector.memset(AT[0:2 * CH, :], 0.0)
        nc.vector.memset(Wcat[0:2 * CH, :], 0.0)
        nc.sync.dma_start(out=wgA_t[:], in_=wgA_p[:])
        nc.sync.dma_start(out=wgB_t[:], in_=wgB_p[:])
        nc.sync.dma_start(out=xg[:], in_=x_grid_p[:])
        nc.sync.dma_start(out=AT[0:CH, :], in_=xT_p[:])
        nc.sync.dma_start(out=idxB0_t[:], in_=idxB0_p[:])
        nc.sync.dma_start(out=idxB1_t[:], in_=idxB1_p[:])
        nc.sync.dma_start(out=idxA0_t[:], in_=idxA0_p[:])
        nc.sync.dma_start(out=idxA1_t[:], in_=idxA1_p[:])
        nc.sync.dma_start(out=idxP_t[:], in_=idxP_p[:])
        nc.sync.dma_start(out=w_id0_t[:], in_=w_id0_p[:])
        nc.sync.dma_start(out=w_id1_t[:], in_=w_id1_p[:])
        nc.sync.dma_start(out=Wcat[2 * CH:4 * CH, :], in_=w_dif_p[:])
        nc.sync.dma_start(out=bias_t[:], in_=bias_p[:])
        nc.sync.dma_start(out=lin_w_t[:], in_=lin_w_p[:])
        nc.sync.dma_start(out=lin_b_t[:], in_=lin_b_p[:])
        make_identity(nc, ident[:])

        nc.vector.tensor_add(out=Wcat[0:CH, :], in0=w_id0_t[:], in1=w_id1_t[:])
        # bias halves: Z-part scaled by 0.5 for the tanh-based sigmoid
        nc.vector.tensor_scalar_mul(out=bias_h[0:FILT, :], in0=bias_t[0:FILT, :],
                                    scalar1=0.5)
        nc.vector.tensor_copy(out=bias_h[FILT:P, :], in_=bias_t[FILT:P, :])

        # ---- phase 1: degrees (both in pi order) ----
        degI = persist.tile([P, T], f32)
        degO = persist.tile([P, T, 1], f32)
        for wg_t, deg, chunks in ((wgA_t, degI[:, :], cfg["chunksAd"]),
                                  (wgB_t, degO[:, :, 0], cfg["chunksBd"])):
            for (t0, t1, o0, o1, runs) in chunks:
                ro = o0
                for (r0, r1, D) in runs:
                    nt = r1 - r0
                    nc.vector.tensor_reduce(
                        out=deg[:, r0:r1],
                        in_=wg_t[:, ro:ro + nt * D].rearrange(
                            "p (t d) -> p t d", t=nt),
                        axis=mybir.AxisListType.X, op=mybir.AluOpType.add)
                    ro += nt * D

        rin = persist.tile([P, T], f32)
        rout = persist.tile([P, T], f32)
        nc.vector.tensor_scalar_max(out=rin[:], in0=degI[:], scalar1=1e-30)
        nc.vector.reciprocal(out=rin[:], in_=rin[:])
        nc.vector.tensor_scalar_max(out=rout[:], in0=degO[:, :, 0], scalar1=1e-30)
        nc.vector.reciprocal(out=rout[:], in_=rout[:])

        # ---- phase 2: xs tables, AllGathers, restrides ----
        xsI = persist.tile([P, T, CH], f32)
        xsO = persist.tile([P, T, CH], f32)
        nc.vector.tensor_tensor(out=xsI[:], in0=xg[:],
                                in1=rin[:].to_broadcast([P, T, CH]),
                                op=mybir.AluOpType.mult)
        nc.sync.dma_start(out=bounceI[:], in_=xsI[:])
        nc.gpsimd.collective_compute(
            "AllGather", mybir.AluOpType.bypass,
            replica_groups=replica_groups,
            ins=[bounceI[:]], outs=[tableI[:]])
        nc.vector.tensor_tensor(out=xsO[:], in0=xg[:],
                                in1=rout[:].to_broadcast([P, T, CH]),
                                op=mybir.AluOpType.mult)
        nc.sync.dma_start(out=bounceO[:], in_=xsO[:])
        nc.gpsimd.collective_compute(
            "AllGather", mybir.AluOpType.bypass,
            replica_groups=replica_groups,
            ins=[bounceO[:]], outs=[tableO[:]])

        # restride [NT,16] -> [NT,64] via SBUF (HWDGE both ways)
        NTP = NT // P  # 392
        for tab, tab64 in ((tableI, tableI64), (tableO, tableO64)):
            rt = rpool.tile([P, NTP, CH], f32, tag="restride")
            nc.sync.dma_start(out=rt[:], in_=tab[:].rearrange(
                "(p w) c -> p w c", p=P))
            nc.scalar.dma_start(out=tab64[:, 0:CH].rearrange(
                "(p w) c -> p w c", p=P), in_=rt[:])

        # ---- phase 3: gather storms + segmented reduces ----
        TxC = persist.tile([P, T, 2 * CH], f32)
        TxIs = persist.tile([P, T, CH], f32)
        Tx1 = persist.tile([P, T, CH], f32)
        qctr = [0]

        def storm(idx_t, chunks, table64, base, out_ap3):
            """Gather one region grid and reduce into out_ap3 [P, T, CH]."""
            view = table64[base:NT, 0:CH] if base else table64[:, 0:CH]
            for (t0, t1, o0, o1, runs) in chunks:
                wchunk = o1 - o0
                G = gpool.tile([P, WMAX, CH], f32, tag="gbuf")
                for c0 in range(0, wchunk, COLS_PER_INST):
                    cw = min(COLS_PER_INST, wchunk - c0)
                    dma_gather_q(
                        G[:, c0:c0 + cw, :], view,
                        idx_t[:, (o0 + c0) * 8:(o0 + c0 + cw) * 8],
                        cw * P, qctr[0] % 4)
                    qctr[0] += 1
                ro = 0
                for (r0, r1, D) in runs:
                    nt = r1 - r0
                    nc.vector.tensor_reduce(
                        out=out_ap3[:, r0:r1, :],
                        in_=G[:, ro:ro + nt * D, :].rearrange(
                            "p (t d) c -> p t c d", t=nt),
                        axis=mybir.AxisListType.X, op=mybir.AluOpType.add)
                    ro += nt * D

        # B direction first (needs tableI): two regions summed into TxIs
        storm(idxB0_t, cfg["chunksB0"], tableI64, 0, TxIs[:])
        storm(idxB1_t, cfg["chunksB1"], tableI64, cut_row, Tx1[:])
        nc.vector.tensor_add(out=TxIs[:], in0=TxIs[:], in1=Tx1[:])
        # permute TxI sigma->pi through strided DRAM + gather
        nc.sync.dma_start(out=txis_d[:, 0:CH], in_=TxIs[:])
        TxIp = persist.tile([P, T, CH], f32)
        for c0 in range(0, T, COLS_PER_INST):
            cw = min(COLS_PER_INST, T - c0)
            dma_gather_q(TxIp[:, c0:c0 + cw, :], txis_d[:, 0:CH],
                         idxP_t[:, c0 * 8:(c0 + cw) * 8], cw * P,
                         qctr[0] % 4)
            qctr[0] += 1
        nc.vector.tensor_copy(out=TxC[:, :, CH:2 * CH], in_=TxIp[:])

        # A direction (needs tableO): regions into TxC[:, :, 0:16]
        storm(idxA0_t, cfg["chunksA0"], tableO64, 0,
              TxC[:, :, 0:CH])
        storm(idxA1_t, cfg["chunksA1"], tableO64, cut_row, Tx1[:])
        nc.vector.tensor_add(out=TxC[:, :, 0:CH], in0=TxC[:, :, 0:CH],
                             in1=Tx1[:])

        # ---- phase 4: transposes into AT rows 32:64 ----
        for g0 in range(0, T, 4):
            nt = min(4, T - g0)
            ps = psum.tile([P, P], f32, tag="tps")
            nc.tensor.transpose(
                out=ps[0:nt * 2 * CH, :],
                in_=TxC[:, g0:g0 + nt, :].rearrange("p t c -> p (t c)"),
                identity=ident[:])
            for i in range(nt):
                nc.scalar.copy(
                    out=AT[2 * CH:4 * CH, (g0 + i) * P:(g0 + i + 1) * P],
                    in_=ps[i * 2 * CH:(i + 1) * 2 * CH, :])

        # ---- phase 5: epilogue ----
        out_sb = persist.tile([1, NL], f32)
        CW = 512
        nchunks = (NL + CW - 1) // CW
        for c in range(nchunks):
            lo = c * CW
            w = min(CW, NL - lo)
            pre = psum_pre.tile([P, CW], f32, tag="pre")
            nc.tensor.matmul(out=pre[:, 0:w], lhsT=Wcat[:], rhs=AT[:, lo:lo + w],
                             start=True, stop=True)
            z = work.tile([FILT, CW], f32, tag="z")
            ht = work.tile([FILT, CW], f32, tag="ht")
            # sigmoid(x) = 0.5*tanh(0.5*x) + 0.5
            nc.scalar.activation(out=z[:, 0:w], in_=pre[0:FILT, 0:w],
                                 func=mybir.ActivationFunctionType.Tanh,
                                 bias=bias_h[0:FILT, :], scale=0.5)
            nc.scalar.activation(out=ht[:, 0:w], in_=pre[FILT:P, 0:w],
                                 func=mybir.ActivationFunctionType.Tanh,
                                 bias=bias_h[FILT:P, :], scale=1.0)
            nc.vector.tensor_scalar(out=z[:, 0:w], in0=z[:, 0:w],
                                    scalar1=0.5, scalar2=0.5,
                                    op0=mybir.AluOpType.mult,
                                    op1=mybir.AluOpType.add)
            h = work.tile([FILT, CW], f32, tag="h")
            nc.vector.tensor_mul(out=h[:, 0:w], in0=z[:, 0:w], in1=ht[:, 0:w])
            nc.vector.tensor_tensor(out=h[:, 0:w], in0=ht[:, 0:w], in1=h[:, 0:w],
                                    op=mybir.AluOpType.subtract)
            nc.vector.tensor_scalar_max(out=h[:, 0:w], in0=h[:, 0:w], scalar1=0.0)
            ps2 = psum.tile([1, CW], f32, tag="ps2")
            nc.tensor.matmul(out=ps2[:, 0:w], lhsT=lin_w_t[:], rhs=h[:, 0:w],
                             start=True, stop=True)
            nc.vector.tensor_scalar_add(out=out_sb[:, lo:lo + w], in0=ps2[:, 0:w],
                                        scalar1=lin_b_t[0:1, :])
        nc.sync.dma_start(out=out_p[:], in_=out_sb[:])

    nc.compile()
    return nc


# ---------------------------------------------------------------------------
# harness entry point
# ---------------------------------------------------------------------------

_CACHE = {}


def kernel(x, edge_index, edge_weight, w_z, b_z, w_r, b_r, w_h, b_h, lin_w, lin_b):
    """Distributed DCRNN forward on 8 TRN2 NeuronCores.

    Takes full unsharded inputs, returns the full [N, 1] float32 output.
    (w_r/b_r are dead inputs: H0 = 0 makes the reset gate a no-op.)
    """
    from concourse.bass_utils import run_bass_kernel_spmd

    x = np.ascontiguousarray(np.asarray(x, dtype=np.float32))
    cfg, arrays = preprocess(x, np.asarray(edge_index), np.asarray(edge_weight),
                             n_cores=8)
    in_maps = make_in_maps(cfg, arrays, np.asarray(w_z, np.float32),
                           np.asarray(b_z, np.float32),
                           np.asarray(w_h, np.float32),
                           np.asarray(b_h, np.float32),
                           np.asarray(lin_w, np.float32),
                           np.asarray(lin_b, np.float32))
    key = (cfg["N"], cfg["E"], cfg["W0A"], cfg["W1A"], cfg["W0B"], cfg["W1B"],
           cfg["WAd"], cfg["WBd"])
    nc = _CACHE.get(key)
    if nc is None:
        nc = build_kernel(cfg)
        _CACHE[key] = nc
    res = run_bass_kernel_spmd(nc, in_maps, core_ids=list(range(8)))
    return postprocess(cfg, arrays, res.results)
